# revision 1
# baseline (speedup 1.0000x reference)
# BiLSTM-CRF NLL loss kernel for Trainium2, 8-core SPMD, batch-parallel.
#
# Sharding: 8 cores x 4 sequences each. Every core runs the full pipeline
# (embedding gather -> fused dense+input projections -> fwd/bwd LSTM ->
# emissions -> CRF) for its 4 sequences and emits a partial scalar; the
# host sums the 8 partials plus the (index-only) gold-path constants.
#
# Device layouts (per core, P = SBUF partition dim):
#   token tau = b*L + t           (b = local sequence, t = time)
#   emb   [128, NTC, EP]          p = tau % 128, chunk = tau // 128
#   embT  [128, KE, NTOK]         p = e % 128   (after PE transpose)
#   xpre  [128, MC, BC, L]        p = gate-unit % 128, gate rows permuted [i,f,o,g]
#   hall  [128, KH, BC, L]        p = h-unit % 128
#   emT   [T, BC, L]              emissions, tag on partition
#   CRF forward scan in exp space: alpha[T, BC], lhsT = exp(trans).

import numpy as np
import ml_dtypes

import concourse.bass as bass
import concourse.mybir as mybir
import concourse.tile as tile
from concourse import bacc
from concourse.bass import IndirectOffsetOnAxis
from concourse.bass_utils import run_bass_kernel_spmd
from concourse.masks import make_identity

F32 = mybir.dt.float32
BF16 = mybir.dt.bfloat16
I32 = mybir.dt.int32
AF = mybir.ActivationFunctionType
OP = mybir.AluOpType

# Real problem dims
REAL = dict(B=32, L=256, VW=100000, VG=100000, DW=300, DG=100, H=256, T=9)
NCORES = 8


def gate_perm(H):
    # reference gate order i,f,g,o -> device order i,f,o,g
    return np.r_[0:H, H:2 * H, 3 * H:4 * H, 2 * H:3 * H]


def build_kernel(cfg):
    """Builds the per-core Bass program. Returns (nc, names) where names maps
    logical tensors to DRAM tensor names."""
    B, L, VW, VG, DW, DG, H, T = (cfg[k] for k in
                                  ("B", "L", "VW", "VG", "DW", "DG", "H", "T"))
    phases = cfg.get("phases", ("gather", "xpre", "lstm", "em", "crf"))
    BC = B // NCORES
    E = DW + DG
    EP = ((E + 127) // 128) * 128          # padded embedding dim (512)
    KE = EP // 128                          # emb K chunks (4)
    GU = 4 * H                              # gate units (1024)
    MC = GU // 128                          # gate-unit chunks (8)
    KH = H // 128                           # h chunks (2)
    NTOK = BC * L
    NTC = NTOK // 128                       # token chunks (8)
    assert NTOK % 128 == 0
    NT_X = (NTOK + 511) // 512              # 512-col chunks for xpre matmul

    nc = bacc.Bacc("TRN2", target_bir_lowering=False, debug=False, num_devices=1)

    # ---- DRAM IO ----
    w2v = nc.dram_tensor("w2v", [VW, DW], F32, kind="ExternalInput")
    glv = nc.dram_tensor("glv", [VG, DG], F32, kind="ExternalInput")
    idw = nc.dram_tensor("idw", [128, NTC], I32, kind="ExternalInput")
    idg = nc.dram_tensor("idg", [128, NTC], I32, kind="ExternalInput")
    weff = {d: nc.dram_tensor(f"weff_{d}", [128, KE, GU], F32, kind="ExternalInput") for d in "fb"}
    beff = {d: nc.dram_tensor(f"beff_{d}", [128, MC], F32, kind="ExternalInput") for d in "fb"}
    whh = {d: nc.dram_tensor(f"whh_{d}", [128, KH, GU], BF16, kind="ExternalInput") for d in "fb"}
    emw = nc.dram_tensor("emw", [128, 2 * KH, T], F32, kind="ExternalInput")
    emb_b = nc.dram_tensor("emb_b", [T, 1], F32, kind="ExternalInput")
    oh = nc.dram_tensor("oh", [T, BC, L], F32, kind="ExternalInput")
    etr = nc.dram_tensor("etr", [T, T], F32, kind="ExternalInput")
    est = nc.dram_tensor("est", [T, 1], F32, kind="ExternalInput")
    een = nc.dram_tensor("een", [T, 1], F32, kind="ExternalInput")
    y = nc.dram_tensor("y", [1, 1], F32, kind="ExternalOutput")

    with tile.TileContext(nc) as tc:
        with tc.tile_pool(name="persist", bufs=1) as pp, \
             tc.tile_pool(name="const", bufs=1) as cp:
            ident = cp.tile([128, 128], F32)
            make_identity(nc, ident[:])

            # persistent SBUF tensors
            sb_xpre = {d: pp.tile([128, MC, BC, L], F32, tag=f"xpre{d}", name=f"xpre{d}") for d in "fb"}
            sb_hall = {d: pp.tile([128, KH, BC, L], F32, tag=f"hall{d}", name=f"hall{d}") for d in "fb"}
            sb_whh = {d: pp.tile([128, KH, GU], BF16, tag=f"whh{d}", name=f"whhsb{d}") for d in "fb"}
            sb_beff = {d: pp.tile([128, MC], F32, tag=f"beff{d}", name=f"beffsb{d}") for d in "fb"}
            sb_emw = pp.tile([128, 2 * KH, T], F32)
            sb_embb = pp.tile([T, 1], F32)
            sb_etr = pp.tile([T, T], F32)
            sb_est = pp.tile([T, 1], F32)
            sb_een = pp.tile([T, 1], F32)
            sb_oh = pp.tile([T, BC, L], F32)
            for d in "fb":
                nc.sync.dma_start(sb_whh[d][:], whh[d][:])
                nc.sync.dma_start(sb_beff[d][:], beff[d][:])
            nc.sync.dma_start(sb_emw[:], emw[:])
            nc.sync.dma_start(sb_embb[:], emb_b[:])
            nc.sync.dma_start(sb_etr[:], etr[:])
            nc.sync.dma_start(sb_est[:], est[:])
            nc.sync.dma_start(sb_een[:], een[:])
            nc.sync.dma_start(sb_oh[:], oh[:])

            # ---- P1: gather + P2: transpose + P3: x_pre ----
            sb_embT = pp.tile([128, KE, NTOK], F32)
            if "gather" not in phases:
                nc.gpsimd.memset(sb_embT[:], 0.0)
            else:
              with tc.tile_pool(name="ph1", bufs=2) as p1, \
                 tc.tile_pool(name="ph1ps", bufs=4, space="PSUM") as p1ps:
                sb_ids = p1.tile([128, 2 * NTC], I32, tag="ids")
                nc.sync.dma_start(sb_ids[:, 0:NTC], idw[:])
                nc.sync.dma_start(sb_ids[:, NTC:2 * NTC], idg[:])
                sb_emb = p1.tile([128, NTC, EP], F32, tag="emb")
                if EP > E:
                    nc.gpsimd.memset(sb_emb[:, :, E:EP], 0.0)
                for n in range(NTC):
                    nc.gpsimd.indirect_dma_start(
                        out=sb_emb[:, n, 0:DW], out_offset=None, in_=w2v[:],
                        in_offset=IndirectOffsetOnAxis(ap=sb_ids[:, n:n + 1], axis=0))
                    nc.gpsimd.indirect_dma_start(
                        out=sb_emb[:, n, DW:E], out_offset=None, in_=glv[:],
                        in_offset=IndirectOffsetOnAxis(ap=sb_ids[:, NTC + n:NTC + n + 1], axis=0))
                # transpose emb -> embT
                for n in range(NTC):
                    for kc in range(KE):
                        pst = p1ps.tile([128, 128], F32, tag="tp")
                        nc.tensor.transpose(pst[:], sb_emb[:, n, kc * 128:(kc + 1) * 128], ident[:])
                        nc.scalar.copy(sb_embT[:, kc, n * 128:(n + 1) * 128], pst[:])

            # x_pre = embT.T @ weff + beff   (unit-major output)
            if "xpre" not in phases:
                for d in "fb":
                    nc.gpsimd.memset(sb_xpre[d][:], 0.0)
            else:
              with tc.tile_pool(name="ph3", bufs=2) as p3, \
                 tc.tile_pool(name="ph3ps", bufs=2, space="PSUM") as p3ps:
                for d in "fb":
                    sb_weff = p3.tile([128, KE, GU], F32, tag="weff")
                    nc.sync.dma_start(sb_weff[:], weff[d][:])
                    xv = sb_xpre[d][:].rearrange("p m b l -> p m (b l)")
                    for mc in range(MC):
                        for nt in range(NT_X):
                            n0, n1 = nt * 512, min((nt + 1) * 512, NTOK)
                            psx = p3ps.tile([128, 512], F32, tag="psx")
                            for kc in range(KE):
                                nc.tensor.matmul(
                                    out=psx[:, 0:n1 - n0],
                                    lhsT=sb_weff[:, kc, mc * 128:(mc + 1) * 128],
                                    rhs=sb_embT[:, kc, n0:n1],
                                    start=(kc == 0), stop=(kc == KE - 1))
                            nc.scalar.activation(xv[:, mc, n0:n1], psx[:, 0:n1 - n0],
                                                 AF.Identity, bias=sb_beff[d][:, mc:mc + 1])

            # ---- P4: the two LSTM recurrences ----
            with tc.tile_pool(name="st", bufs=1) as stp, \
                 tc.tile_pool(name="lt", bufs=3) as ltp, \
                 tc.tile_pool(name="ltps", bufs=4, space="PSUM") as ltps:
                h_bf = {d: stp.tile([128, KH, BC], BF16, tag=f"hbf{d}", name=f"hbf{d}") for d in "fb"}
                c_st = {d: stp.tile([128, KH, BC], F32, tag=f"c{d}", name=f"cst{d}") for d in "fb"}
                for d in "fb":
                    nc.gpsimd.memset(h_bf[d][:], 0.0)
                    nc.gpsimd.memset(c_st[d][:], 0.0)
                if "lstm" not in phases:
                    for d in "fb":
                        nc.gpsimd.memset(sb_hall[d][:], 0.0)
                for t in (range(L) if "lstm" in phases else []):
                    for d in "fb":
                        tt = t if d == "f" else L - 1 - t
                        psg = ltps.tile([128, MC, BC], F32, tag="psg")
                        for mc in range(MC):
                            for kc in range(KH):
                                nc.tensor.matmul(
                                    out=psg[:, mc, :],
                                    lhsT=sb_whh[d][:, kc, mc * 128:(mc + 1) * 128],
                                    rhs=h_bf[d][:, kc, :],
                                    start=(kc == 0), stop=(kc == KH - 1))
                        g = ltp.tile([128, MC, BC], F32, tag="g")
                        nc.vector.tensor_add(g[:], psg[:], sb_xpre[d][:, :, :, tt])
                        S = ltp.tile([128, 3 * KH, BC], F32, tag="S")
                        Tg = ltp.tile([128, KH, BC], F32, tag="Tg")
                        nc.scalar.activation(S[:], g[:, 0:3 * KH, :], AF.Sigmoid)
                        nc.scalar.activation(Tg[:], g[:, 3 * KH:4 * KH, :], AF.Tanh)
                        u = ltp.tile([128, KH, BC], F32, tag="u")
                        nc.vector.tensor_mul(u[:], S[:, 0:KH, :], Tg[:])
                        nc.vector.tensor_mul(c_st[d][:], S[:, KH:2 * KH, :], c_st[d][:])
                        nc.vector.tensor_add(c_st[d][:], c_st[d][:], u[:])
                        tc_ = ltp.tile([128, KH, BC], F32, tag="tc")
                        nc.scalar.activation(tc_[:], c_st[d][:], AF.Tanh)
                        hf32 = ltp.tile([128, KH, BC], F32, tag="hf32")
                        nc.vector.tensor_mul(hf32[:], S[:, 2 * KH:3 * KH, :], tc_[:])
                        nc.vector.tensor_copy(h_bf[d][:], hf32[:])
                        nc.scalar.copy(sb_hall[d][:, :, :, tt], hf32[:])

            # ---- P5: emissions  emT[T, BC, L] ----
            sb_emT = pp.tile([T, BC, L], F32)
            if "em" not in phases:
                nc.gpsimd.memset(sb_emT[:], 0.0)
            else:
              with tc.tile_pool(name="ph5ps", bufs=2, space="PSUM") as p5ps:
                emv = sb_emT[:].rearrange("T b l -> T (b l)")
                for nt in range(NT_X):
                    n0, n1 = nt * 512, min((nt + 1) * 512, NTOK)
                    pse = p5ps.tile([T, 512], F32, tag="pse")
                    k = 0
                    for di, d in enumerate("fb"):
                        hv = sb_hall[d][:].rearrange("p k b l -> p k (b l)")
                        for kc in range(KH):
                            nc.tensor.matmul(out=pse[:, 0:n1 - n0],
                                             lhsT=sb_emw[:, di * KH + kc, :],
                                             rhs=hv[:, kc, n0:n1],
                                             start=(k == 0), stop=(k == 2 * KH - 1))
                            k += 1
                    nc.scalar.activation(emv[:, n0:n1], pse[:, 0:n1 - n0],
                                         AF.Identity, bias=sb_embb[:, 0:1])

            # ---- P6/P7/P8: CRF ----
            with tc.tile_pool(name="crf", bufs=2) as cfp, \
                 tc.tile_pool(name="crfs", bufs=1) as cfs, \
                 tc.tile_pool(name="crfps", bufs=1, space="PSUM") as cfps:
                # gold emission sum (device part of the numerator)
                scr = cfp.tile([T, BC, L], F32, tag="scr")
                acc = cfs.tile([T, 1], F32)
                nc.vector.scalar_tensor_tensor(out=scr[:], in0=sb_emT[:], scalar=1.0,
                                               in1=sb_oh[:], op0=OP.mult, op1=OP.mult,
                                               accum_out=acc[:])
                onesT = cfs.tile([T, 1], F32)
                nc.gpsimd.memset(onesT[:], 1.0)
                ones1T = cfs.tile([1, T], F32)
                nc.gpsimd.memset(ones1T[:], 1.0)
                ps11 = cfps.tile([1, 1], F32, tag="ps11")
                nc.tensor.matmul(out=ps11[:], lhsT=acc[:], rhs=onesT[:], start=True, stop=True)
                emgold = cfs.tile([1, 1], F32)
                nc.vector.tensor_copy(emgold[:], ps11[:])

                # forward algorithm in exp space
                eem = cfs.tile([T, BC, L], F32)
                nc.scalar.activation(eem[:], sb_emT[:], AF.Exp)
                alpha = cfs.tile([T, BC], F32)
                nc.scalar.activation(alpha[:], eem[:, :, 0], AF.Copy, scale=sb_est[:, 0:1])
                logz = cfs.tile([1, BC], F32)
                nc.gpsimd.memset(logz[:], 0.0)
                for t in (range(1, L) if "crf" in phases else []):
                    psa = cfps.tile([T, BC], F32, tag="psa")
                    nc.tensor.matmul(out=psa[:], lhsT=sb_etr[:], rhs=alpha[:], start=True, stop=True)
                    nc.vector.tensor_mul(alpha[:], psa[:], eem[:, :, t])
                    if t % 8 == 7 or t == L - 1:
                        pss = cfps.tile([1, BC], F32, tag="pss")
                        nc.tensor.matmul(out=pss[:], lhsT=onesT[:], rhs=alpha[:], start=True, stop=True)
                        ssum = cfp.tile([1, BC], F32, tag="ssum")
                        nc.vector.tensor_copy(ssum[:], pss[:])
                        rs = cfp.tile([1, BC], F32, tag="rs")
                        nc.vector.reciprocal(rs[:], ssum[:])
                        ls = cfp.tile([1, BC], F32, tag="ls")
                        nc.scalar.activation(ls[:], ssum[:], AF.Ln)
                        nc.vector.tensor_add(logz[:], logz[:], ls[:])
                        psb = cfps.tile([T, BC], F32, tag="psb")
                        nc.tensor.matmul(out=psb[:], lhsT=ones1T[:], rhs=rs[:], start=True, stop=True)
                        nc.vector.tensor_mul(alpha[:], alpha[:], psb[:])
                # den_b = log(sum_t' alpha * exp(end)) + logz
                aen = cfp.tile([T, BC], F32, tag="aen")
                nc.scalar.activation(aen[:], alpha[:], AF.Copy, scale=sb_een[:, 0:1])
                psf = cfps.tile([1, BC], F32, tag="psf")
                nc.tensor.matmul(out=psf[:], lhsT=onesT[:], rhs=aen[:], start=True, stop=True)
                den = cfp.tile([1, BC], F32, tag="den")
                nc.scalar.activation(den[:], psf[:], AF.Ln)
                nc.vector.tensor_add(den[:], den[:], logz[:])
                dsum = cfp.tile([1, 1], F32, tag="dsum")
                nc.vector.tensor_reduce(dsum[:], den[:], axis=mybir.AxisListType.X, op=OP.add)
                res = cfp.tile([1, 1], F32, tag="res")
                nc.vector.tensor_sub(res[:], dsum[:], emgold[:])
                nc.sync.dma_start(y[:], res[:])

    nc.compile()
    return nc


def prep_inputs(cfg, inputs):
    """Host-side prep: fold dense into W_ih, permute gates, build per-core
    input maps and the host-side gold-path constants."""
    B, L, DW, DG, H, T = (cfg[k] for k in ("B", "L", "DW", "DG", "H", "T"))
    BC = B // NCORES
    E = DW + DG
    EP = ((E + 127) // 128) * 128
    KE = EP // 128
    GU = 4 * H
    MC = GU // 128
    KH = H // 128
    NTOK = BC * L
    NTC = NTOK // 128

    f32 = np.float32
    perm = gate_perm(H)
    dense_W = np.asarray(inputs["dense_W"], f32)
    dense_b = np.asarray(inputs["dense_b"], f32)
    shared = {}
    for d, wi, bi, wh in (("f", "W_ih_f", "b_f", "W_hh_f"), ("b", "W_ih_b", "b_b", "W_hh_b")):
        W_ih = np.asarray(inputs[wi], f32)
        b_ = np.asarray(inputs[bi], f32)
        W_eff = (W_ih @ dense_W)[perm]                     # [GU, E]
        b_eff = (W_ih @ dense_b + b_)[perm]                # [GU]
        W_effp = np.zeros((GU, EP), f32)
        W_effp[:, :E] = W_eff
        # lhsT tiles: weff[p, kc, mc*128+m] = W_effp.T[kc*128+p, mc*128+m]
        shared[f"weff_{d}"] = np.ascontiguousarray(
            W_effp.T.reshape(KE, 128, MC, 128).transpose(1, 0, 2, 3).reshape(128, KE, GU))
        shared[f"beff_{d}"] = np.ascontiguousarray(b_eff.reshape(MC, 128).T)
        W_hhp = np.asarray(inputs[wh], f32)[perm]          # [GU, H]
        shared[f"whh_{d}"] = np.ascontiguousarray(
            W_hhp.T.reshape(KH, 128, MC, 128).transpose(1, 0, 2, 3).reshape(128, KH, GU)
        ).astype(ml_dtypes.bfloat16)
    emit_W = np.asarray(inputs["emit_W"], f32)             # [T, 2H]
    shared["emw"] = np.ascontiguousarray(emit_W.T.reshape(2 * KH, 128, T).transpose(1, 0, 2))
    shared["emb_b"] = np.asarray(inputs["emit_b"], f32).reshape(T, 1)
    trans = np.asarray(inputs["crf_trans"], f32)
    start = np.asarray(inputs["crf_start"], f32)
    end = np.asarray(inputs["crf_end"], f32)
    shared["etr"] = np.exp(trans)
    shared["est"] = np.exp(start).reshape(T, 1)
    shared["een"] = np.exp(end).reshape(T, 1)
    shared["w2v"] = np.asarray(inputs["w2v_table"], f32)
    shared["glv"] = np.asarray(inputs["glove_table"], f32)

    wids = np.asarray(inputs["word2vec_ids"], np.int32)
    gids = np.asarray(inputs["glove_ids"], np.int32)
    tags = np.asarray(inputs["input_labels"], np.int64)

    in_maps = []
    host_consts = np.zeros(NCORES, np.float64)
    for c in range(NCORES):
        m = dict(shared)
        sl = slice(c * BC, (c + 1) * BC)
        m["idw"] = np.ascontiguousarray(wids[sl].reshape(NTOK).reshape(NTC, 128).T)
        m["idg"] = np.ascontiguousarray(gids[sl].reshape(NTOK).reshape(NTC, 128).T)
        tg = tags[sl]                                       # [BC, L]
        ohc = np.zeros((T, BC, L), f32)
        ohc[tg, np.arange(BC)[:, None], np.arange(L)[None, :]] = 1.0
        m["oh"] = ohc
        # host gold-path constants (index-only parts of the numerator)
        hc = start[tg[:, 0]].sum() + end[tg[:, -1]].sum()
        hc += trans[tg[:, :-1], tg[:, 1:]].sum()
        host_consts[c] = hc
        in_maps.append(m)
    return in_maps, host_consts


_CACHE = {}


def _get_compiled(key, cfg):
    if key not in _CACHE:
        _CACHE[key] = build_kernel(cfg)
    return _CACHE[key]


def kernel(**inputs):
    cfg = dict(REAL)
    masks = np.asarray(inputs["input_masks"])
    assert masks.min() == 1, "kernel assumes all-ones input_masks"
    nc = _get_compiled("real", cfg)
    in_maps, host_consts = prep_inputs(cfg, inputs)
    res = run_bass_kernel_spmd(nc, in_maps, list(range(NCORES)))
    total = 0.0
    for c in range(NCORES):
        total += float(res.results[c]["y"].ravel()[0]) - host_consts[c]
    return np.float32(total)

